# revision 3
# baseline (speedup 1.0000x reference)
"""Trainium2 Bass kernel for nn_CNOLReLu: bicubic 2x upsample -> leaky_relu
-> antialiased bicubic 2x downsample on a (16,128,128,128) NHWC tensor.

Data-parallel over batch: 2 images per NeuronCore.  Per channel c the op is
Y = D @ f(U @ X @ U.T) @ D.T with X = x[b,:,:,c], U = 128->256 bicubic,
D = 256->128 antialiased bicubic, f = leaky_relu(0.01).  Four matmul hops
(ping-pong layouts, no transposes), processed in 4-channel groups so the
PSUM->SBUF evacuations (the bottleneck: only DVE+ACT can read PSUM) run as
few, large-FD instructions balanced across both engines:
  A: pA[w,  (c4,h2)]  = X_c^T @ U^T      FD-1024 evac on DVE
  B: pZ_t[w2,(c4,h2)] = U_t  @ sP        FD-1024 Lrelu evac on ACT (x2)
  C: pS[w2m,(c4,m,h')]= sA^T @ D_t^T     FD-1024 evac on DVE (banded 8-tap D)
  D: pY[h', (c4,w')]  = D_m @ sS         FD-512 evac on ACT
PSUM bank budget (8): pA 2 + pZ 2x2 (pY reuses a pZ rotation) + pS 2.
"""
import numpy as np
import ml_dtypes
from contextlib import ExitStack

import concourse.bacc as bacc
import concourse.tile as tile
from concourse import mybir
from concourse.bass_utils import run_bass_kernel_spmd

F32 = mybir.dt.float32
BF16 = mybir.dt.bfloat16
AF = mybir.ActivationFunctionType

N_CORES = 8
B_CORE = 2          # images per core
H = W = C = 128
NEG_SLOPE = 0.01


def _keys_cubic(x):
    x = np.abs(x)
    return np.where(
        x <= 1, (1.5 * x - 2.5) * x * x + 1,
        np.where(x < 2, ((-0.5 * x + 2.5) * x - 4) * x + 2, 0.0))


def _resize_matrix(n_in, n_out):
    """Row-stochastic bicubic (antialias) resize operator; matches
    jax.image.resize(method='bicubic', antialias=True)."""
    scale = n_out / n_in
    pos = (np.arange(n_out) + 0.5) / scale - 0.5
    kscale = min(scale, 1.0)
    w = _keys_cubic((np.arange(n_in)[None, :] - pos[:, None]) * kscale)
    return (w / w.sum(axis=1, keepdims=True)).astype(np.float64)


def _band(Dm, t):
    rows = np.nonzero(np.abs(Dm[:, t * 128:(t + 1) * 128]).sum(1) > 0)[0]
    return int(rows.min()), int(rows.max()) + 1


_CACHE = {}


def _build():
    if "nc" in _CACHE:
        return _CACHE["nc"], _CACHE["consts"]

    U = _resize_matrix(H, 2 * H)       # [256,128]
    Dm = _resize_matrix(2 * H, H)      # [128,256]
    uT = U.T.astype(ml_dtypes.bfloat16)                              # [128,256]
    dT = np.concatenate([Dm.T[0:128, :], Dm.T[128:256, :]], axis=1)  # [128,256]
    dT_bf = dT.astype(ml_dtypes.bfloat16)
    bands = [_band(Dm, 0), _band(Dm, 1)]   # [(0,66),(62,128)]

    nc = bacc.Bacc()
    x_d = nc.declare_dram_parameter("x", [B_CORE, H, W, C], BF16, isOutput=False)
    ut_d = nc.declare_dram_parameter("ut", [128, 256], BF16, isOutput=False)
    dbf_d = nc.declare_dram_parameter("dbf", [128, 256], BF16, isOutput=False)
    y_d = nc.declare_dram_parameter("y", [B_CORE, H, W, C], BF16, isOutput=True)

    with tile.TileContext(nc) as tc, ExitStack() as ctx:
        wpool = ctx.enter_context(tc.tile_pool(name="weights", bufs=1))
        xpool = ctx.enter_context(tc.tile_pool(name="ximg", bufs=2))
        opool = ctx.enter_context(tc.tile_pool(name="oimg", bufs=2))
        sppool = ctx.enter_context(tc.tile_pool(name="sP", bufs=2))
        sapool = ctx.enter_context(tc.tile_pool(name="sA", bufs=2))
        sspool = ctx.enter_context(tc.tile_pool(name="sS", bufs=2))
        papool = ctx.enter_context(tc.tile_pool(name="pA", bufs=1, space="PSUM"))
        pzpool = ctx.enter_context(tc.tile_pool(name="pZ", bufs=2, space="PSUM"))
        pspool = ctx.enter_context(tc.tile_pool(name="pS", bufs=1, space="PSUM"))

        ut_s = wpool.tile([128, 256], BF16, tag="ut")
        dbf_s = wpool.tile([128, 256], BF16, tag="dbf")
        nc.sync.dma_start(ut_s[:], ut_d[:])
        nc.sync.dma_start(dbf_s[:], dbf_d[:])

        for b in range(B_CORE):
            ximg = xpool.tile([128, W * C], BF16, tag="ximg")
            nc.sync.dma_start(ximg[:], x_d[b].rearrange("h w c -> h (w c)"))
            oimg = opool.tile([128, W * C], BF16, tag="oimg")

            for g in range(C // 4):          # 4-channel groups
                c0 = g * 4
                # ---- A: pA[:, c*256+(h2)] = X_c^T @ U^T, c in group
                pA = papool.tile([128, 1024], F32, tag="pA")
                for c in range(4):
                    nc.tensor.matmul(pA[:, c * 256:(c + 1) * 256],
                                     ximg[:, (c0 + c)::C], ut_s[:],
                                     start=True, stop=True)
                sP = sppool.tile([128, 1024], BF16, tag="sP")
                nc.vector.tensor_copy(sP[:], pA[:])

                # ---- B: pZ_t = U_chunk_t @ sP ; Lrelu evac per t (ACT)
                sA = sapool.tile([128, 2048], BF16, tag="sA")
                for t in range(2):
                    pZ = pzpool.tile([128, 1024], F32, tag="pZ")
                    for hh in range(2):
                        nc.tensor.matmul(pZ[:, hh * 512:(hh + 1) * 512],
                                         ut_s[:, t * 128:(t + 1) * 128],
                                         sP[:, hh * 512:(hh + 1) * 512],
                                         start=True, stop=True)
                    nc.scalar.activation(sA[:, t * 1024:(t + 1) * 1024],
                                         pZ[:], AF.Lrelu, alpha=NEG_SLOPE)

                # ---- C: banded H'-down (bf16).
                # pS[w2m, c*256 + m*128 + h'] accumulated over t chunks
                pS = pspool.tile([128, 1024], F32, tag="pS")
                for c in range(4):
                    for m in range(2):
                        for t in range(2):
                            lo, hi = bands[t]
                            nc.tensor.matmul(
                                pS[:, c * 256 + m * 128 + lo:
                                   c * 256 + m * 128 + hi],
                                sA[:, t * 1024 + c * 256 + m * 128:
                                   t * 1024 + c * 256 + (m + 1) * 128],
                                dbf_s[:, t * 128 + lo:t * 128 + hi],
                                start=(t == 0), stop=(t == 1),
                                skip_group_check=True)
                sS = sspool.tile([128, 1024], BF16, tag="sS")
                nc.vector.tensor_copy(sS[:], pS[:])

                # ---- D: pY[h', c*128+w'] = sum_m D_m @ sS slice
                pYt = pzpool.tile([128, 1024], F32, tag="pZ")
                pY = pYt[:, 0:512]
                for c in range(4):
                    for m in range(2):
                        nc.tensor.matmul(
                            pY[:, c * 128:(c + 1) * 128],
                            dbf_s[:, m * 128:(m + 1) * 128],
                            sS[:, c * 256 + m * 128:c * 256 + (m + 1) * 128],
                            start=(m == 0), stop=(m == 1),
                            skip_group_check=True)
                # ---- evac pY (c4,w') -> oimg cols w'*C + c, c in group (ACT)
                dsto = oimg[:].rearrange(
                    "h (w c) -> h w c", c=C)[:, :, c0:c0 + 4]
                srco = pY.rearrange("h (c w) -> h w c", c=4)
                nc.scalar.copy(dsto, srco)

            nc.sync.dma_start(y_d[b].rearrange("h w c -> h (w c)"), oimg[:])

    nc.compile()
    consts = {"ut": np.ascontiguousarray(uT),
              "dbf": np.ascontiguousarray(dT_bf)}
    _CACHE["nc"] = nc
    _CACHE["consts"] = consts
    return nc, consts


def kernel(x, in_size=128, out_size=128, trace=False, tmpdir=None):
    x = np.asarray(x, dtype=np.float32)
    assert x.shape == (16, H, W, C), x.shape
    nc, consts = _build()
    in_maps = []
    for core in range(N_CORES):
        m = {"x": np.ascontiguousarray(
            x[core * B_CORE:(core + 1) * B_CORE]).astype(ml_dtypes.bfloat16)}
        m.update(consts)
        in_maps.append(m)
    res = run_bass_kernel_spmd(nc, in_maps, list(range(N_CORES)), trace=trace,
                               tmpdir=tmpdir)
    out = np.concatenate([res.results[i]["y"] for i in range(N_CORES)], axis=0)
    if trace:
        kernel.last_exec_time_ns = res.exec_time_ns
        kernel.last_results = res
    return out.astype(np.float32)
